# revision 1
# baseline (speedup 1.0000x reference)
"""Trainium2 Bass kernel for nn_DecoderBlock (B=4, T=S=1024, DM=1024, H=16, HID=4096).

Sharding: sequence-parallel over T across 8 cores. Core i owns query/token
chunk t in [128*i, 128*(i+1)) for all 4 batches (512 rows, b-major). All
per-token ops (projections, LayerNorm, FFN, residuals) are local; the only
communication is 4 bf16 AllGathers for self/cross attention K^T and V.

Layouts:
  - Activations are feature-major in SBUF: 8 tiles [128 dm, 512 rows] f32r,
    rows = b*128 + t_loc. f32r matmuls at N=512 run at full PE rate with no
    cast passes (weights DMA'd straight into f32r tiles).
  - Attention inner products run in bf16: K^T/Q^T produced feature-major
    [depth, tokens], V token-major [tokens, depth]; scores are computed
    transposed (S^T [kpos, q]) so the AV matmul consumes exp(S^T) directly.
  - Softmax denominator via a ones-vector matmul accumulated next to AV;
    normalization by broadcast-matmul of the reciprocal.
"""
import contextlib
import sys

sys.path.insert(0, "/opt/trn_rl_repo")

import numpy as np

import concourse.bass as bass
import concourse.mybir as mybir
import concourse.tile as tile
from concourse import bacc
from concourse.bass_utils import run_bass_kernel_spmd
from concourse.masks import make_identity

F32 = mybir.dt.float32
F32R = mybir.dt.float32r
BF16 = mybir.dt.bfloat16
AF = mybir.ActivationFunctionType
ALU = mybir.AluOpType

N_CORES = 8
B, T, DM, H, HID = 4, 1024, 1024, 16, 4096
DEPTH = DM // H            # 64
TLOC = T // N_CORES        # 128 tokens per core
ROWS = B * TLOC            # 512 rows per core (b-major)
P = 128
NKT = DM // P              # 8 feature tiles
NHT = HID // P             # 32 hidden tiles
NEG = -1e9
VW = H * (DEPTH + 1)      # V bounce width: 65 cols per head (last is ones)

_CACHE = {}

def _tile(pool, shape, dtype, tag, **kw):
    return pool.tile(shape, dtype, name=tag, tag=tag, **kw)



def _emit(nc, tc, D):
    """Build the whole decoder block inside a TileContext."""
    es = contextlib.ExitStack()
    D["_es"] = es

    def pool(name, **kw):
        return es.enter_context(tc.tile_pool(name=name, **kw))

    scoped = {}

    def pool_open(name, **kw):
        cm = tc.tile_pool(name=name, **kw)
        scoped[name] = cm
        return cm.__enter__()

    def pool_close(name):
        scoped.pop(name).__exit__(None, None, None)

    const = pool("const", bufs=1)
    wpool = pool("wpool", bufs=8)       # streamed weight tiles [128,512] f32r
    spool = pool("spool", bufs=3)        # misc staging
    epool = pool("epool", bufs=3)        # exp(S^T) tiles
    dram = pool("dram", bufs=1, space="DRAM")
    pp = pool("pp", bufs=8, space="PSUM")

    # ---- constants -------------------------------------------------------
    id_r = _tile(const, [P, P], F32, "id_r")
    make_identity(nc, id_r[:])
    id_b = _tile(const, [P, P], BF16, "id_b")
    make_identity(nc, id_b[:])
    ones_col_f = _tile(const, [P, 1], F32, "ones_col_f")
    nc.vector.memset(ones_col_f[:], 1.0)
    ones_col_r = _tile(const, [P, 1], F32R, "ones_col_r")
    nc.vector.tensor_copy(out=ones_col_r[:], in_=ones_col_f[:])
    ones_col_b = _tile(const, [P, 1], BF16, "ones_col_b")
    nc.vector.memset(ones_col_b[:], 1.0)
    ones_row_f = _tile(const, [1, P], F32, "ones_row_f")
    nc.vector.memset(ones_row_f[:], 1.0)
    ones_row_r = _tile(const, [1, P], F32R, "ones_row_r")
    nc.vector.tensor_copy(out=ones_row_r[:], in_=ones_row_f[:])
    ones_row_b = _tile(const, [1, P], BF16, "ones_row_b")
    nc.vector.memset(ones_row_b[:], 1.0)
    ones_sb16 = _tile(const, [P, H], BF16, "ones_sb16")
    nc.vector.memset(ones_sb16[:], 1.0)
    eps_t = _tile(const, [1, 1], F32, "eps_t")
    nc.vector.memset(eps_t[:], 1e-6)

    def vec_tiles(name, n=NKT, scale=None):
        """DRAM [n*128] vector -> n SBUF [128,1] f32 tiles."""
        v = D[name]
        out = []
        for j in range(n):
            t = _tile(const, [P, 1], F32, f"{name}_{j}")
            nc.sync.dma_start(t[:], v[j * P:(j + 1) * P][:, None])
            if scale is not None:
                nc.vector.tensor_scalar_mul(t[:], t[:], scale)
            out.append(t)
        return out

    def row_halves(name):
        """DRAM [1024] -> two [1, 512] f32r tiles (free-dim bias rows)."""
        v = D[name]
        out = []
        for g in range(2):
            t = _tile(const, [1, 512], F32R, f"{name}_row{g}")
            nc.sync.dma_start(t[:], v[g * 512:(g + 1) * 512][None, :].bitcast(F32R))
            out.append(t)
        return out

    bQ1 = vec_tiles("bq1", scale=0.125)
    bK1 = vec_tiles("bk1")
    bV1 = row_halves("bv1")
    bO1 = vec_tiles("bo1")
    bQ2 = vec_tiles("bq2", scale=0.125)
    bK2 = vec_tiles("bk2")
    bV2 = row_halves("bv2")
    bO2 = vec_tiles("bo2")
    bH = vec_tiles("bh", n=NHT)
    bOUT = vec_tiles("bout")
    G1, BE1 = vec_tiles("g1"), vec_tiles("be1")
    G2, BE2 = vec_tiles("g2"), vec_tiles("be2")
    G3, BE3 = vec_tiles("g3"), vec_tiles("be3")

    # ---- entry transposes: [4,128,1024] f32 token-major -> 8 x [128,512] f32r
    def entry_T(src, tagp, tpool):
        outs = [_tile(tpool, [P, ROWS], F32R, f"{tagp}{j}") for j in range(NKT)]
        for b in range(B):
            row = _tile(spool, [P, DM], F32, "entry_row", bufs=1)
            nc.sync.dma_start(row[:], src[b])
            for j in range(NKT):
                ps = _tile(pp, [P, P], F32, "ps")
                nc.tensor.transpose(ps[:], row[:, j * P:(j + 1) * P], id_r[:])
                nc.scalar.activation(outs[j][:, b * P:(b + 1) * P], ps[:], AF.Copy)
        return outs

    p_pre = pool_open("p_pre", bufs=1)
    p_ao = pool_open("p_ao", bufs=1)
    kpool = pool_open("kpool", bufs=1)   # gathered K tiles per b
    vpool = pool_open("vpool", bufs=1)   # gathered V tiles per b
    p_x = pool_open("p_x", bufs=1)
    xT = entry_T(D["xq"], "xT", p_x)

    # ---- mask prep: maskq [4,128,1024] -> maskT [4][2] of [128,512] bf16*(-1e9)
    maskT = []
    for b in range(B):
        row = _tile(spool, [P, T], F32, "mask_row", bufs=1)
        nc.sync.dma_start(row[:], D["maskq"][b])
        mrow = _tile(spool, [P, T], BF16, "mask_scaled", bufs=1)
        nc.vector.tensor_scalar_mul(mrow[:], row[:], NEG)
        gtiles = []
        for g in range(2):
            mt = _tile(const, [P, 512], BF16, f"maskT_{b}_{g}")
            for c in range(4):
                j = 4 * g + c
                ps = _tile(pp, [P, P], BF16, "ps")
                nc.tensor.transpose(ps[:], mrow[:, j * P:(j + 1) * P], id_b[:])
                nc.scalar.activation(mt[:, c * P:(c + 1) * P], ps[:], AF.Copy)
            gtiles.append(mt)
        maskT.append(gtiles)

    # ---- generic projection helpers -------------------------------------
    def load_w_tiles(wname, kt, g):
        """Weight tile [128k, 512 dout] f32r from DRAM [K, N]."""
        t = _tile(wpool, [P, 512], F32R, "w")
        nc.sync.dma_start(t[:], D[wname][kt * P:(kt + 1) * P,
                                         g * 512:(g + 1) * 512].bitcast(F32R))
        return t

    def proj_feature_major(wname, actT, evict):
        """out^T[dout, rows] = w^T @ act^T ; evict(psum, d) per dout tile.

        k-outer loop: 4 dout psums accumulate in parallel while weight
        tiles stream through a small ring (each tile read 4x then freed).
        """
        ng = {"wh": 8}.get(wname, 2)
        nkt = {"wout": NHT}.get(wname, NKT)
        for g in range(ng):
            pss = [_tile(pp, [P, ROWS], F32, "ps") for _ in range(4)]
            for k in range(nkt):
                wt = load_w_tiles(wname, k, g)
                for c in range(4):
                    nc.tensor.matmul(pss[c][:], wt[:, c * P:(c + 1) * P],
                                     actT[k][:], start=(k == 0),
                                     stop=(k == nkt - 1))
            for c in range(4):
                evict(pss[c], 4 * g + c)

    def proj_token_major(wname, actT, brow, bounce):
        """V = act @ w + b, token-major [rows, dout]; DMA into bounce DRAM."""
        for g in range(2):
            pss = [_tile(pp, [P, 512], F32, "ps") for _ in range(4)]
            for k in range(NKT):
                wt = load_w_tiles(wname, k, g)
                for r in range(4):
                    nc.tensor.matmul(pss[r][:], actT[k][:, r * P:(r + 1) * P],
                                     wt[:], start=(k == 0), stop=False)
            for r in range(4):
                nc.tensor.matmul(pss[r][:], ones_row_r[:, :P], brow[g][:],
                                 start=False, stop=True)
                sb = _tile(spool, [P, 512], BF16, "v_evict", bufs=2)
                nc.scalar.activation(sb[:], pss[r][:], AF.Copy)
                dst = bounce[:].rearrange("r (h c) -> r h c", c=DEPTH + 1)[
                    r * P:(r + 1) * P, g * 8:(g + 1) * 8, 0:DEPTH]
                nc.sync.dma_start(
                    dst, sb[:].rearrange("p (h c) -> p h c", c=DEPTH))
        for r in range(4):
            nc.sync.dma_start(
                bounce[:].rearrange("r (h c) -> r h c", c=DEPTH + 1)[
                    r * P:(r + 1) * P, :, DEPTH:DEPTH + 1],
                ones_sb16[:, :, None])

    # ---- K/V projections + AllGathers (issued as early as possible) ------
    def kv_and_ag(actT, wk_name, wv_name, bk, bv, tagp):
        k_in = _tile(dram, [DM, ROWS], BF16, f"{tagp}k_in")
        v_in = _tile(dram, [ROWS, VW], BF16, f"{tagp}v_in")
        k_g = _tile(dram, [N_CORES * DM, ROWS], BF16, f"{tagp}k_g", addr_space="Shared")
        v_g = _tile(dram, [N_CORES * ROWS, VW], BF16, f"{tagp}v_g", addr_space="Shared")

        def evict_k(ps, d):
            sb = _tile(spool, [P, ROWS], BF16, "k_evict", bufs=2)
            nc.scalar.activation(sb[:], ps[:], AF.Identity, bias=bk[d][:])
            nc.sync.dma_start(k_in[d * P:(d + 1) * P, :], sb[:])

        proj_feature_major(wk_name, actT, evict_k)
        nc.gpsimd.collective_compute(
            "AllGather", ALU.bypass,
            replica_groups=[list(range(N_CORES))],
            ins=[k_in[:].opt()], outs=[k_g[:].opt()])
        proj_token_major(wv_name, actT, bv, v_in)
        nc.gpsimd.collective_compute(
            "AllGather", ALU.bypass,
            replica_groups=[list(range(N_CORES))],
            ins=[v_in[:].opt()], outs=[v_g[:].opt()])
        return k_g, v_g

    k1g, v1g = kv_and_ag(xT, "wk1", "wv1", bK1, bV1, "s")

    p_enc = pool_open("p_enc", bufs=1)
    encT = entry_T(D["enc"], "encT", p_enc)
    k2g, v2g = kv_and_ag(encT, "wk2", "wv2", bK2, bV2, "c")
    pool_close("p_enc")

    # ---- Q projection -> bf16 feature-major tiles ------------------------
    def q_proj(wname, actT, bq, tagp, tpool):
        qT = [_tile(tpool, [P, ROWS], BF16, f"{tagp}{j}") for j in range(NKT)]

        def evict_q(ps, d):
            nc.scalar.activation(qT[d][:], ps[:], AF.Identity,
                                 bias=bq[d][:], scale=0.125)
        proj_feature_major(wname, actT, evict_q)
        return qT

    p_q1 = pool_open("p_q1", bufs=1)
    q1T = q_proj("wq1", xT, bQ1, "q1T", p_q1)

    # ---- attention core --------------------------------------------------
    def attention(qT, k_g, v_g, masked, aoT):
        for b in range(B):
            ktiles = []   # [j][p] -> [128,128] bf16 (kpos-tile j, dm-tile p)
            for j in range(N_CORES):
                tj = []
                for p in range(NKT):
                    t = _tile(kpool, [P, P], BF16, f"kt{j}_{p}")
                    nc.sync.dma_start(
                        t[:], k_g[j * DM + p * P: j * DM + (p + 1) * P,
                                  b * P:(b + 1) * P])
                    tj.append(t)
                ktiles.append(tj)
            vtiles = []
            for j in range(N_CORES):
                t = _tile(vpool, [P, VW], BF16, f"vt{j}")
                nc.sync.dma_start(
                    t[:], v_g[j * ROWS + b * P: j * ROWS + (b + 1) * P, :])
                vtiles.append(t)
            for h in range(H):
                hp, ho = h // 2, (h % 2) * DEPTH
                qs = qT[hp][ho:ho + DEPTH, b * P:(b + 1) * P]
                exps = []
                for g in range(2):
                    ps = _tile(pp, [P, 512], F32, "ps")
                    for c in range(4):
                        j = 4 * g + c
                        nc.tensor.matmul(
                            ps[:, c * P:(c + 1) * P],
                            ktiles[j][hp][ho:ho + DEPTH, :], qs,
                            start=True, stop=True)
                    if masked:
                        nc.vector.tensor_add(ps[:], ps[:], maskT[b][g][:])
                    ex = _tile(epool, [P, 512], BF16, "expS")
                    nc.scalar.activation(ex[:], ps[:], AF.Exp)
                    exps.append(ex)
                av = _tile(pp, [DEPTH + 1, P], F32, "ps")
                for g in range(2):
                    for c in range(4):
                        j = 4 * g + c
                        nc.tensor.matmul(
                            av[:], vtiles[j][:, h * (DEPTH + 1):(h + 1) * (DEPTH + 1)],
                            exps[g][:, c * P:(c + 1) * P],
                            start=(j == 0), stop=(j == N_CORES - 1))
                recip = _tile(spool, [1, P], F32, "recip")
                nc.vector.reciprocal(recip[:], av[DEPTH:DEPTH + 1, :])
                bcs = _tile(spool, [DEPTH, P], F32, "bcast_sb")
                nc.gpsimd.partition_broadcast(bcs[:], recip[:])
                nc.vector.tensor_mul(aoT[hp][ho:ho + DEPTH, b * P:(b + 1) * P],
                                     av[0:DEPTH, :], bcs[:])

    aoT = [_tile(p_ao, [P, ROWS], F32R, f"aoT{j}") for j in range(NKT)]
    attention(q1T, k1g, v1g, True, aoT)
    pool_close("p_q1")

    # ---- out-projection + residual + LN ---------------------------------
    def layer_norm(vT, G, BE, out_dtype, tagp, tpool):
        """Feature-major LN over dm (partition axis) via ones-matmuls."""
        s_ps = _tile(pp, [1, ROWS], F32, "ps")
        q_ps = _tile(pp, [1, ROWS], F32, "ps")
        for k in range(NKT):
            nc.tensor.matmul(s_ps[:], ones_col_r[:], vT[k][:],
                             start=(k == 0), stop=(k == NKT - 1))
        for k in range(NKT):
            sq = _tile(spool, [P, ROWS], F32R, "ln_sq", bufs=2)
            nc.vector.tensor_mul(sq[:], vT[k][:], vT[k][:])
            nc.tensor.matmul(q_ps[:], ones_col_r[:], sq[:],
                             start=(k == 0), stop=(k == NKT - 1))
        mean = _tile(spool, [1, ROWS], F32, "ln_mean")
        nc.vector.tensor_scalar_mul(mean[:], s_ps[:], 1.0 / DM)
        ex2 = _tile(spool, [1, ROWS], F32, "ln_ex2")
        nc.vector.tensor_scalar_mul(ex2[:], q_ps[:], 1.0 / DM)
        var = _tile(spool, [1, ROWS], F32, "ln_var")
        nc.vector.scalar_tensor_tensor(var[:], mean[:], -1.0, mean[:],
                                       op0=ALU.mult, op1=ALU.mult)
        nc.vector.tensor_add(var[:], var[:], ex2[:])
        std = _tile(spool, [1, ROWS], F32, "ln_std")
        nc.scalar.activation(std[:], var[:], AF.Sqrt, bias=eps_t[:])
        rstd = _tile(spool, [1, ROWS], F32R, "ln_rstd")
        with nc.allow_low_precision(reason="f32r rstd keeps full f32 bits"):
            nc.vector.reciprocal(rstd[:], std[:])
        nm = _tile(spool, [1, ROWS], F32R, "ln_nm")
        nc.vector.scalar_tensor_tensor(nm[:], mean[:], -1.0, rstd[:],
                                       op0=ALU.mult, op1=ALU.mult)
        r_ps = _tile(pp, [P, ROWS], F32, "ps")
        nc.tensor.matmul(r_ps[:], ones_row_r[:], rstd[:], start=True, stop=True)
        n_ps = _tile(pp, [P, ROWS], F32, "ps")
        nc.tensor.matmul(n_ps[:], ones_row_r[:], nm[:], start=True, stop=True)
        outs = []
        for k in range(NKT):
            tmp = _tile(spool, [P, ROWS], F32, "ln_tmp", bufs=2)
            nc.vector.tensor_mul(tmp[:], vT[k][:], r_ps[:])
            nc.vector.tensor_add(tmp[:], tmp[:], n_ps[:])
            o = _tile(tpool, [P, ROWS], out_dtype, f"{tagp}{k}")
            nc.scalar.activation(o[:], tmp[:], AF.Identity,
                                 bias=BE[k][:], scale=G[k][:])
            outs.append(o)
        return outs

    def out_proj_resid(wname, inT, bo, residT, tagp, tpool):
        vT = []
        def evict(ps, d):
            o = _tile(tpool, [P, ROWS], F32R, f"pre{d}")
            nc.vector.scalar_tensor_tensor(o[:], ps[:], bo[d][:], residT[d][:],
                                           op0=ALU.add, op1=ALU.add)
            vT.append(o)
        proj_feature_major(wname, inT, evict)
        return vT

    v1 = out_proj_resid("wo1", aoT, bO1, xT, "h1pre", p_pre)
    pool_close("p_x")
    p_h1 = pool_open("p_h1", bufs=1)
    h1T = layer_norm(v1, G1, BE1, F32R, "h1T", p_h1)

    # ---- cross attention -------------------------------------------------
    p_q2 = pool_open("p_q2", bufs=1)
    q2T = q_proj("wq2", h1T, bQ2, "q2T", p_q2)
    attention(q2T, k2g, v2g, False, aoT)
    pool_close("p_q2")
    v2 = out_proj_resid("wo2", aoT, bO2, h1T, "h2pre", p_pre)
    pool_close("p_h1")
    pool_close("vpool")
    pool_close("kpool")
    pool_close("p_ao")
    p_h2 = pool_open("p_h2", bufs=1)
    h2T = layer_norm(v2, G2, BE2, F32R, "h2T", p_h2)

    # ---- FFN -------------------------------------------------------------
    p_u = pool_open("p_u", bufs=1)
    uT = [None] * NHT
    def evict_u(ps, d):
        t = _tile(p_u, [P, ROWS], F32R, f"uT{d}")
        nc.scalar.activation(t[:], ps[:], AF.Relu, bias=bH[d][:])
        uT[d] = t
    proj_feature_major("wh", h2T, evict_u)

    v3 = out_proj_resid("wout", uT, bOUT, h2T, "fpre", p_pre)
    pool_close("p_u")
    p_o = pool_open("p_o", bufs=1)
    oT = layer_norm(v3, G3, BE3, F32, "oT", p_o)

    # ---- exit transpose + store -----------------------------------------
    for j in range(NKT):
        for b in range(B):
            ps = _tile(pp, [P, P], F32, "ps")
            nc.tensor.transpose(ps[:], oT[j][:, b * P:(b + 1) * P], id_r[:])
            sb = _tile(spool, [P, P], F32, "out_sb", bufs=2)
            nc.scalar.activation(sb[:], ps[:], AF.Copy)
            nc.sync.dma_start(D["out"][b][:, j * P:(j + 1) * P], sb[:])
    for name in reversed(list(scoped)):
        scoped.pop(name).__exit__(None, None, None)


def _close_rest(scoped):
    for name in reversed(list(scoped)):
        scoped.pop(name).__exit__(None, None, None)


def build():
    if "nc" in _CACHE:
        return _CACHE["nc"]
    nc = bacc.Bacc("TRN2", target_bir_lowering=False, debug=False,
                   enable_asserts=True, num_devices=N_CORES)
    D = {}
    def inp(name, shape):
        D[name] = nc.dram_tensor(name, list(shape), F32, kind="ExternalInput").ap()
    inp("xq", (B, TLOC, DM))
    inp("enc", (B, TLOC, DM))
    inp("maskq", (B, TLOC, T))
    for w in ["wq1", "wk1", "wv1", "wo1", "wq2", "wk2", "wv2", "wo2"]:
        inp(w, (DM, DM))
        inp("b" + w[1:], (DM,))
    inp("wh", (DM, HID))
    inp("bh", (HID,))
    inp("wout", (HID, DM))
    inp("bout", (DM,))
    for i in (1, 2, 3):
        inp(f"g{i}", (DM,))
        inp(f"be{i}", (DM,))
    D["out"] = nc.dram_tensor("out", [B, TLOC, DM], F32,
                              kind="ExternalOutput").ap()
    with tile.TileContext(nc) as tc:
        _emit(nc, tc, D)
        D["_es"].close()
    nc.compile()
    _CACHE["nc"] = nc
    return nc


def _make_in_maps(inputs):
    x = np.ascontiguousarray(inputs["x"], dtype=np.float32)
    enc = np.ascontiguousarray(inputs["enc_out"], dtype=np.float32)
    mask = np.ascontiguousarray(inputs["look_ahead_mask"], dtype=np.float32)
    shared = {}
    for w in ["wq1", "wk1", "wv1", "wo1", "wq2", "wk2", "wv2", "wo2"]:
        shared[w] = np.ascontiguousarray(inputs[w], dtype=np.float32)
        shared["b" + w[1:]] = np.ascontiguousarray(inputs["b" + w[1:]],
                                                   dtype=np.float32)
    shared["wh"] = np.ascontiguousarray(inputs["wh"], dtype=np.float32)
    shared["bh"] = np.ascontiguousarray(inputs["bh"], dtype=np.float32)
    shared["wout"] = np.ascontiguousarray(inputs["wout"], dtype=np.float32)
    shared["bout"] = np.ascontiguousarray(inputs["bout"], dtype=np.float32)
    for i in (1, 2, 3):
        shared[f"g{i}"] = np.ascontiguousarray(inputs[f"g{i}"], dtype=np.float32)
        shared[f"be{i}"] = np.ascontiguousarray(inputs[f"be{i}"], dtype=np.float32)
    in_maps = []
    for i in range(N_CORES):
        sl = slice(i * TLOC, (i + 1) * TLOC)
        m = dict(shared)
        m["xq"] = np.ascontiguousarray(x[:, sl, :])
        m["enc"] = np.ascontiguousarray(enc[:, sl, :])
        m["maskq"] = np.ascontiguousarray(mask[:, 0, sl, :])
        in_maps.append(m)
    return in_maps


def _assemble(res):
    out = np.empty((B, T, DM), dtype=np.float32)
    for i in range(N_CORES):
        out[:, i * TLOC:(i + 1) * TLOC, :] = res.results[i]["out"]
    return out


def kernel(**inputs):
    nc = build()
    in_maps = _make_in_maps(inputs)
    res = run_bass_kernel_spmd(nc, in_maps, core_ids=list(range(N_CORES)))
    return _assemble(res)



# revision 25
# speedup vs baseline: 1.4997x; 1.4997x over previous
"""Trainium2 Bass kernel for nn_DecoderBlock (B=4, T=S=1024, DM=1024, H=16, HID=4096).

Sharding: sequence-parallel over T across 8 cores. Core i owns query/token
chunk t in [128*i, 128*(i+1)) for all 4 batches (512 rows, b-major). All
per-token ops (projections, LayerNorm, FFN, residuals) are local; the only
communication is one bf16 AllGather per attention carrying both K^T and V.

v2 design notes (packet-count driven — DMA queues cost ~70ns/packet):
  - Host pre-work (not on the device clock): weights cast to bf16, x/enc
    pre-transposed to feature-major, causal mask pre-transposed/scaled to
    the [kpos, (chunk, q)] bf16 tiles the kernel consumes, all 1-D params
    packed into one [17, 1024] matrix, output returned feature-major and
    transposed back on host.
  - All weight DMAs are [128, 2KB-row] bf16 tiles; K/V gather read back as
    [128, 512]/[128, 1040] tiles (1-2KB rows) shared across all 4 batches.
  - K and V travel in a single flat AllGather buffer per attention.
  - Softmax: exp(S^T) with ones-column folded into V for the denominator;
    normalization batched at the end of attention (one reciprocal [16,512],
    one selector-matmul broadcast + one multiply per feature tile).
  - Scores for the two heads of a feature tile run concurrently on the PE
    (disjoint 64-row groups -> implicit row tiling).
"""
import contextlib
import sys

sys.path.insert(0, "/opt/trn_rl_repo")

import numpy as np
import ml_dtypes

import concourse.bass as bass
import concourse.mybir as mybir
import concourse.tile as tile
from concourse import bacc
from concourse.bass_utils import run_bass_kernel_spmd
from concourse.masks import make_identity

F32 = mybir.dt.float32
F32R = mybir.dt.float32r
BF16 = mybir.dt.bfloat16
AF = mybir.ActivationFunctionType
ALU = mybir.AluOpType
BF = ml_dtypes.bfloat16

N_CORES = 8
B, T, DM, H, HID = 4, 1024, 1024, 16, 4096
DEPTH = DM // H            # 64
TLOC = T // N_CORES        # 128 tokens per core
ROWS = B * TLOC            # 512 rows per core (b-major)
P = 128
NKT = DM // P              # 8 feature tiles

# packed 1-D params: rows of the [NV, 1024] "vecs" input
VQ1, VK1, VO1, VQ2, VK2, VO2, VOUT = 0, 1, 2, 3, 4, 5, 6
VG1, VBE1, VG2, VBE2, VG3, VBE3 = 7, 8, 9, 10, 11, 12
VBH0 = 13                  # bh occupies rows 13..16
NV = 17

KBE = DM * ROWS            # K elems in a kv gather block
VW = 2 * 520               # V row: 2 g-blocks of 8 heads x (64 depth + 1 one)
VBE = ROWS * VW
BLK = KBE + VBE

_CACHE = {}
DEBUG_DUMPS = False
STAGE = 99


def _tile(pool, shape, dtype, tag, **kw):
    return pool.tile(shape, dtype, name=tag, tag=tag, **kw)


def _emit(nc, tc, D):
    es = contextlib.ExitStack()
    D["_es"] = es

    def pool(name, **kw):
        return es.enter_context(tc.tile_pool(name=name, **kw))

    const = pool("const", bufs=1)
    wpool = pool("wpool", bufs=3 if DEBUG_DUMPS else 4)
    spool = pool("spool", bufs=3)        # misc staging
    epool = pool("epool", bufs=4 if DEBUG_DUMPS else 6)
    dram = pool("dram", bufs=1, space="DRAM")
    pp = pool("pp", bufs=8, space="PSUM")

    if STAGE <= -3:
        t_early = _tile(const, [P, ROWS], BF16, "bfa0")
        nc.sync.dma_start(t_early[:], D["xTb"].bitcast(BF16)[0:P, :])
        if DEBUG_DUMPS:
            nc.sync.dma_start(D["dbg_echo"], D["wq1"].bitcast(BF16)[0:P, 0:1024])
            nc.sync.dma_start(D["dbg_xb"], t_early[:])
        return

    # ---- constants -------------------------------------------------------
    id_f = _tile(const, [P, P], F32, "id_f")
    make_identity(nc, id_f[:])
    ones_col_f = _tile(const, [P, 1], F32, "ones_col_f")
    nc.vector.memset(ones_col_f[:], 1.0)
    ones_col_r = _tile(const, [P, 1], F32R, "ones_col_r")
    nc.vector.tensor_copy(out=ones_col_r[:], in_=ones_col_f[:])
    ones_row_f = _tile(const, [1, P], F32, "ones_row_f")
    nc.vector.memset(ones_row_f[:], 1.0)
    ones_row_r = _tile(const, [1, P], F32R, "ones_row_r")
    nc.vector.tensor_copy(out=ones_row_r[:], in_=ones_row_f[:])
    ones_row_b = _tile(const, [1, P], BF16, "ones_row_b")
    nc.vector.memset(ones_row_b[:], 1.0)
    eps_t = _tile(const, [1, 1], F32, "eps_t")
    nc.vector.memset(eps_t[:], 1e-6)

    if STAGE <= -2:
        t_early = _tile(const, [P, ROWS], BF16, "bfa0")
        nc.sync.dma_start(t_early[:], D["xTb"].bitcast(BF16)[0:P, :])
        if DEBUG_DUMPS:
            nc.sync.dma_start(D["dbg_echo"], D["wq1"].bitcast(BF16)[0:P, 0:1024])
            nc.sync.dma_start(D["dbg_xb"], t_early[:])
        return

    # head-pair selector for the softmax-normalization broadcast matmul
    # (host-built: sel[r, hp*128 + p] = 1 iff r == 2*hp + (p >= 64)).
    # NOTE: loaded as f32 + on-device copy — a dma_start from a .bitcast(F32R)
    # DRAM view corrupts adjacent input DRAM (f32r cast-DMA overrun).
    sel_f = _tile(const, [16, NKT * P], F32, "sel_f")
    nc.sync.dma_start(sel_f[:], D["sel"])
    sel_r = _tile(const, [16, NKT * P], F32R, "sel_r")
    nc.vector.tensor_copy(out=sel_r[:], in_=sel_f[:])

    # ---- packed 1-D params: one DMA + 8 PE transposes --------------------
    vec_sb = _tile(const, [NV, DM], F32, "vec_sb")
    nc.sync.dma_start(vec_sb[:], D["vecs"])
    bias_sb = _tile(const, [P, NKT * NV], F32, "bias_sb")
    for j in range(NKT):
        ps = _tile(pp, [P, NV], F32, "ps")
        nc.tensor.transpose(ps[:], vec_sb[:, j * P:(j + 1) * P],
                            id_f[0:NV, 0:NV])
        nc.scalar.activation(bias_sb[:, j * NV:(j + 1) * NV], ps[:], AF.Copy)

    def bvec(v, j):
        return bias_sb[:, j * NV + v:j * NV + v + 1]

    # pre-scaled q biases (activation computes f(x*scale + bias))
    bq_s = _tile(const, [P, 16], F32, "bq_s")
    bias3 = bias_sb[:].rearrange("p (j v) -> p j v", v=NV)
    nc.vector.tensor_scalar_mul(bq_s[:, 0:8], bias3[:, :, VQ1], 0.125)
    nc.vector.tensor_scalar_mul(bq_s[:, 8:16], bias3[:, :, VQ2], 0.125)

    def bq_ap(mha, j):
        return bq_s[:, mha * 8 + j:mha * 8 + j + 1]

    if STAGE <= -1:
        t_early = _tile(const, [P, ROWS], BF16, "bfa0")
        nc.sync.dma_start(t_early[:], D["xTb"].bitcast(BF16)[0:P, :])
        if DEBUG_DUMPS:
            nc.sync.dma_start(D["dbg_echo"], D["wq1"].bitcast(BF16)[0:P, 0:1024])
            nc.sync.dma_start(D["dbg_xb"], t_early[:])
        return

    # V bias rows (free-dim layout), bf16 halves from host
    brow = [[None, None], [None, None]]
    for mha in range(2):
        for g in range(2):
            t = _tile(const, [1, 512], BF16, f"brow{mha}_{g}")
            nc.sync.dma_start(t[:], D["bvb"].bitcast(BF16)[mha, g][None, :])
            brow[mha][g] = t

    # causal mask, host-prepared as [kpos, (chunk c, q)] * -1e9, bf16
    maskT = []
    for g in range(2):
        t = _tile(const, [P, 512], BF16, f"maskT{g}")
        nc.sync.dma_start(t[:], D["maskT"].bitcast(BF16)[g])
        maskT.append(t)

    # ---- flat phase pools (lifetimes managed by tag reuse) ----------------
    p_bfa = pool("p_bfa", bufs=1)    # xTb -> h1Tb
    p_bfb = pool("p_bfb", bufs=1)    # encTb -> h2Tb
    p_q = pool("p_q", bufs=1)        # q1T -> q2T
    p_aob = pool("p_aob", bufs=1)    # aoTb + den
    kpool = pool("kpool", bufs=1)    # kt tiles; uT reuses 32 of its slots
    vpool = pool("vpool", bufs=1)    # vt tiles
    p_acc = pool("p_acc", bufs=1)    # aoU/v-pre-LN/oT, all via acc tags

    # ---- activations in, feature-major (bf16; residuals ride bf16 too) ----
    xTb = []
    for p in range(NKT):
        tb = _tile(p_bfa, [P, ROWS], BF16, f"bfa{p}")
        nc.sync.dma_start(tb[:], D["xTb"].bitcast(BF16)[p * P:(p + 1) * P, :])
        xTb.append(tb)

    if DEBUG_DUMPS:
        nc.sync.dma_start(D["dbg_echo"], D["wq1"].bitcast(BF16)[0:P, 0:1024])
        nc.sync.dma_start(D["dbg_xb"], xTb[0][:])
    if STAGE <= 0:
        return
        dwt = _tile(spool, [P, 1024], BF16, "dbgw", bufs=1)
        nc.sync.dma_start(dwt[:], D["wq1"].bitcast(BF16)[0:P, 0:1024])
        nc.sync.dma_start(D["dbg_w"], dwt[:])

    # ---- projection helpers ----------------------------------------------
    def wtile(wname, k, col0):
        t = _tile(wpool, [P, 1024], BF16, "w")
        nc.sync.dma_start(
            t[:], D[wname].bitcast(BF16)[k * P:(k + 1) * P, col0:col0 + 1024])
        return t

    def proj8(wname, act, evict, nkt=NKT, col0=0):
        """8 psums [128,512] accumulate over k; weight tiles [128,1024]."""
        pss = [_tile(pp, [P, ROWS], F32, "ps") for _ in range(8)]
        for k in range(nkt):
            wt = wtile(wname, k, col0)
            for s in range(8):
                nc.tensor.matmul(pss[s][:], wt[:, s * P:(s + 1) * P],
                                 act[k][:], start=(k == 0),
                                 stop=(k == nkt - 1))
        for s in range(8):
            evict(pss[s], s)

    def proj_v(wname, act, mha, vdst):
        """V = act @ w + b, token-major, evicted to the kv bounce buffer
        as [rows, g-block of 8*(64+1)] with the ones column for the softmax
        denominator memset in SBUF (no tiny DMA writes)."""
        pss = [_tile(pp, [P, 512], F32, "ps") for _ in range(8)]
        for k in range(NKT):
            wt = wtile(wname, k, 0)
            for g in range(2):
                for r in range(4):
                    nc.tensor.matmul(pss[g * 4 + r][:],
                                     act[k][:, r * P:(r + 1) * P],
                                     wt[:, g * 512:(g + 1) * 512],
                                     start=(k == 0), stop=False)
        for g in range(2):
            for r in range(4):
                ps = pss[g * 4 + r]
                nc.tensor.matmul(ps[:], ones_row_b[:], brow[mha][g][:],
                                 start=False, stop=True)
                sb = _tile(spool, [P, 520], BF16, "v_evict", bufs=2)
                sb3 = sb[:].rearrange("p (h c) -> p h c", c=65)
                nc.scalar.activation(
                    sb3[:, :, 0:64],
                    ps[:].rearrange("p (h c) -> p h c", c=64), AF.Copy)
                nc.vector.memset(sb3[:, :, 64:65], 1.0)
                nc.sync.dma_start(
                    vdst[r * P:(r + 1) * P, g * 520:(g + 1) * 520], sb[:])

    # ---- K/V projections + combined AllGather ----------------------------
    def kv_and_ag(act, wkn, wvn, mha, tag):
        kv_in = _tile(dram, [BLK], BF16, f"{tag}kv_in")
        kv_g = _tile(dram, [N_CORES * BLK], BF16, f"{tag}kv_g",
                     addr_space="Shared")
        kdst = kv_in[0:KBE].rearrange("(a b) -> a b", b=ROWS)
        vdst = kv_in[KBE:BLK].rearrange("(a b) -> a b", b=VW)

        def evict_k(ps, s):
            sb = _tile(spool, [P, ROWS], BF16, "k_evict", bufs=2)
            nc.scalar.activation(sb[:], ps[:], AF.Identity,
                                 bias=bvec(VK1 if mha == 0 else VK2, s))
            nc.sync.dma_start(kdst[s * P:(s + 1) * P, :], sb[:])

        proj8(wkn, act, evict_k)
        proj_v(wvn, act, mha, vdst)
        nc.gpsimd.collective_compute(
            "AllGather", ALU.bypass,
            replica_groups=[list(range(N_CORES))],
            ins=[kv_in[:].opt()], outs=[kv_g[:].opt()])
        return kv_g

    kv1_g = kv_and_ag(xTb, "wk1", "wv1", 0, "s")
    if STAGE <= 1:
        return

    encTb = []
    for p in range(NKT):
        t = _tile(p_bfb, [P, ROWS], BF16, f"bfb{p}")
        nc.sync.dma_start(t[:], D["encTb"].bitcast(BF16)[p * P:(p + 1) * P, :])
        encTb.append(t)
    kv2_g = kv_and_ag(encTb, "wk2", "wv2", 1, "c")

    # ---- Q projection -> bf16 feature-major tiles ------------------------
    def q_proj(wname, act, mha, tagp, tpool):
        qT = [_tile(tpool, [P, ROWS], BF16, f"{tagp}{s}") for s in range(8)]

        def evict_q(ps, s):
            nc.scalar.activation(qT[s][:], ps[:], AF.Identity,
                                 bias=bq_ap(mha, s), scale=0.125)
        proj8(wname, act, evict_q)
        return qT

    q1T = q_proj("wq1", xTb, 0, "qT", p_q)

    # ---- attention core --------------------------------------------------
    def attention(qT, kv_g, masked, aoTb):
        dstg_d = _tile(dram, [B, 16, P], F32, "dstg_d")
        kt = [[_tile(kpool, [P, ROWS], BF16, f"kt{j}_{p}") for p in range(8)]
              for j in range(8)]
        for j in range(8):
            base = j * BLK
            for p in range(8):
                nc.sync.dma_start(
                    kt[j][p][:],
                    kv_g[base + p * P * ROWS: base + (p + 1) * P * ROWS]
                    .rearrange("(a b) -> a b", b=ROWS))
        aoU = [_tile(p_acc, [P, ROWS], F32, f"acc{s}") for s in range(8)]
        den = _tile(p_aob, [16, ROWS], F32, "den")
        for b in range(B):
            stg = _tile(spool, [1, 16 * P], F32, "dstg_sb", bufs=2)
            vt = []
            for j in range(8):
                t = _tile(vpool, [P, VW], BF16, f"vt{j}")
                nc.sync.dma_start(
                    t[:],
                    kv_g[j * BLK + KBE + b * P * VW:
                         j * BLK + KBE + (b + 1) * P * VW]
                    .rearrange("(a b) -> a b", b=VW))
                vt.append(t)
            for hp in range(NKT):
                exs = [[None, None], [None, None]]   # [hh][g]
                for g in range(2):
                    pshh = [_tile(pp, [P, 512], F32, "ps") for _ in range(2)]
                    for c in range(4):
                        j = 4 * g + c
                        for hh in range(2):
                            ho = hh * DEPTH
                            nc.tensor.matmul(
                                pshh[hh][:, c * P:(c + 1) * P],
                                kt[j][hp][ho:ho + DEPTH, b * P:(b + 1) * P],
                                qT[hp][ho:ho + DEPTH, b * P:(b + 1) * P],
                                start=True, stop=True)
                    for hh in range(2):
                        if masked:
                            nc.vector.tensor_add(pshh[hh][:], pshh[hh][:],
                                                 maskT[g][:])
                        ex = _tile(epool, [P, 512], BF16, "expS")
                        nc.scalar.activation(ex[:], pshh[hh][:], AF.Exp)
                        exs[hh][g] = ex
                avs = [_tile(pp, [DEPTH + 1, P], F32, "ps")
                       for _ in range(2)]
                for g in range(2):
                    for c in range(4):
                        j = 4 * g + c
                        for hh in range(2):
                            h = 2 * hp + hh
                            gv, hv = h // 8, h % 8
                            nc.tensor.matmul(
                                avs[hh][:],
                                vt[j][:, gv * 520 + hv * 65:
                                      gv * 520 + (hv + 1) * 65],
                                exs[hh][g][:, c * P:(c + 1) * P],
                                start=(j == 0), stop=(j == 7))
                for hh in range(2):
                    h = 2 * hp + hh
                    nc.scalar.activation(
                        aoU[hp][hh * DEPTH:(hh + 1) * DEPTH,
                                b * P:(b + 1) * P],
                        avs[hh][0:DEPTH, :], AF.Copy)
                    nc.vector.tensor_copy(
                        out=stg[0:1, h * P:(h + 1) * P],
                        in_=avs[hh][DEPTH:DEPTH + 1, :])
            nc.sync.dma_start(dstg_d[b].rearrange("h q -> (h q)")[None, :],
                              stg[:])
        nc.sync.dma_start(den[:].rearrange("h (b q) -> h b q", q=P),
                          dstg_d[:].rearrange("b h q -> h b q"))
        if DEBUG_DUMPS and masked:
            nc.sync.dma_start(D["dbg_q"], qT[0][:])
            nc.sync.dma_start(D["dbg_kt"], kt[0][0][:])
            nc.sync.dma_start(D["dbg_vt"], vt[7][:])
            nc.sync.dma_start(D["dbg_ex"], exs[0][0][:])
            nc.sync.dma_start(D["dbg_den"], den[:])
            nc.sync.dma_start(D["dbg_ao"], aoU[0][:])
        recip = _tile(spool, [16, ROWS], F32R, "recip", bufs=1)
        with nc.allow_low_precision(reason="f32r recip keeps full f32 bits"):
            nc.vector.reciprocal(recip[:], den[:])
        for hp in range(NKT):
            sc = _tile(pp, [P, ROWS], F32, "ps")
            nc.tensor.matmul(sc[:], sel_r[:, hp * P:(hp + 1) * P], recip[:],
                             start=True, stop=True)
            nc.vector.tensor_mul(aoTb[hp][:], aoU[hp][:], sc[:])
        if DEBUG_DUMPS and masked:
            nc.sync.dma_start(D["dbg_aob"], aoTb[0][:])

    aoTb = [_tile(p_aob, [P, ROWS], BF16, f"aoTb{s}") for s in range(8)]
    attention(q1T, kv1_g, True, aoTb)
    if STAGE <= 2:
        return

    # ---- out-projection + residual + LN ----------------------------------
    def out_proj_resid(wname, inT, vb, residT, tagp, tpool):
        vT = []

        def evict(ps, s):
            o = _tile(tpool, [P, ROWS], F32R, f"acc{s}")
            nc.vector.scalar_tensor_tensor(o[:], ps[:], bvec(vb, s),
                                           residT[s][:],
                                           op0=ALU.add, op1=ALU.add)
            vT.append(o)
        proj8(wname, inT, evict, nkt=len(inT))
        return vT

    def layer_norm(vT, vg, vbe, out_dtype, tagp, tpool):
        """Feature-major LN over dm (partition axis) via ones-matmuls."""
        s_ps = _tile(pp, [1, ROWS], F32, "ps")
        q_ps = _tile(pp, [1, ROWS], F32, "ps")
        for k in range(NKT):
            nc.tensor.matmul(s_ps[:], ones_col_r[:], vT[k][:],
                             start=(k == 0), stop=(k == NKT - 1))
        for k in range(NKT):
            sq = _tile(spool, [P, ROWS], F32R, "ln_sq", bufs=2)
            nc.vector.tensor_mul(sq[:], vT[k][:], vT[k][:])
            nc.tensor.matmul(q_ps[:], ones_col_r[:], sq[:],
                             start=(k == 0), stop=(k == NKT - 1))
        mean = _tile(spool, [1, ROWS], F32, "ln_mean", bufs=1)
        nc.vector.tensor_scalar_mul(mean[:], s_ps[:], 1.0 / DM)
        ex2 = _tile(spool, [1, ROWS], F32, "ln_ex2", bufs=1)
        nc.vector.tensor_scalar_mul(ex2[:], q_ps[:], 1.0 / DM)
        var = _tile(spool, [1, ROWS], F32, "ln_var", bufs=1)
        nc.vector.scalar_tensor_tensor(var[:], mean[:], -1.0, mean[:],
                                       op0=ALU.mult, op1=ALU.mult)
        nc.vector.tensor_add(var[:], var[:], ex2[:])
        std = _tile(spool, [1, ROWS], F32, "ln_std", bufs=1)
        nc.scalar.activation(std[:], var[:], AF.Sqrt, bias=eps_t[:])
        rstd = _tile(spool, [1, ROWS], F32R, "ln_rstd", bufs=1)
        with nc.allow_low_precision(reason="f32r rstd keeps full f32 bits"):
            nc.vector.reciprocal(rstd[:], std[:])
        nm = _tile(spool, [1, ROWS], F32R, "ln_nm", bufs=1)
        nc.vector.scalar_tensor_tensor(nm[:], mean[:], -1.0, rstd[:],
                                       op0=ALU.mult, op1=ALU.mult)
        r_ps = _tile(pp, [P, ROWS], F32, "ps")
        nc.tensor.matmul(r_ps[:], ones_row_r[:], rstd[:], start=True,
                         stop=True)
        n_ps = _tile(pp, [P, ROWS], F32, "ps")
        nc.tensor.matmul(n_ps[:], ones_row_r[:], nm[:], start=True, stop=True)
        outs = []
        for k in range(NKT):
            tmp = _tile(spool, [P, ROWS], F32, "ln_tmp", bufs=2)
            nc.vector.tensor_mul(tmp[:], vT[k][:], r_ps[:])
            nc.vector.tensor_add(tmp[:], tmp[:], n_ps[:])
            o = _tile(tpool, [P, ROWS], out_dtype, f"{tagp}{k}")
            nc.scalar.activation(o[:], tmp[:], AF.Identity,
                                 bias=bvec(vbe, k), scale=bvec(vg, k))
            outs.append(o)
        return outs

    v1 = out_proj_resid("wo1", aoTb, VO1, xTb, "h1pre", p_acc)
    h1Tb = layer_norm(v1, VG1, VBE1, BF16, "bfa", p_bfa)

    # ---- cross attention -------------------------------------------------
    q2T = q_proj("wq2", h1Tb, 1, "qT", p_q)
    attention(q2T, kv2_g, False, aoTb)
    v2 = out_proj_resid("wo2", aoTb, VO2, h1Tb, "h2pre", p_acc)
    h2Tb = layer_norm(v2, VG2, VBE2, BF16, "bfb", p_bfb)

    # ---- FFN (uT reuses kt slots: kt dead after attn2) --------------------
    uT = [None] * 32

    def mk_evict_u(g2):
        def ev(ps, s):
            d = g2 * 8 + s
            t = _tile(kpool, [P, ROWS], BF16, f"kt{d // 8}_{d % 8}")
            nc.scalar.activation(t[:], ps[:], AF.Relu,
                                 bias=bvec(VBH0 + d // 8, d % 8))
            uT[d] = t
        return ev

    for g2 in range(4):
        proj8("wh", h2Tb, mk_evict_u(g2), col0=g2 * 1024)

    v3 = out_proj_resid("wout", uT, VOUT, h2Tb, "fpre", p_acc)
    oT = layer_norm(v3, VG3, VBE3, F32, "acc", p_acc)

    # ---- store (feature-major; host transposes back) ---------------------
    for s in range(NKT):
        nc.sync.dma_start(D["out"][s * P:(s + 1) * P, :], oT[s][:])
    if DEBUG_DUMPS:
        nc.sync.dma_start(D["dbg_echo2"], D["wq1"].bitcast(BF16)[0:P, 0:1024])


def build():
    if "nc" in _CACHE:
        return _CACHE["nc"]
    nc = bacc.Bacc("TRN2", target_bir_lowering=False, debug=False,
                   enable_asserts=True, num_devices=N_CORES)
    D = {}

    def inp(name, shape, dtype=F32):
        D[name] = nc.dram_tensor(name, list(shape), dtype,
                                 kind="ExternalInput").ap()
    U32 = mybir.dt.uint32
    inp("xTb", (DM, ROWS // 2), U32)
    inp("encTb", (DM, ROWS // 2), U32)
    inp("maskT", (2, P, 256), U32)
    inp("vecs", (NV, DM))
    inp("sel", (16, NKT * P))
    inp("bvb", (2, 2, 256), U32)
    for w in ["wq1", "wk1", "wv1", "wo1", "wq2", "wk2", "wv2", "wo2"]:
        inp(w, (DM, DM // 2), U32)
    inp("wh", (DM, HID // 2), U32)
    inp("wout", (HID, DM // 2), U32)
    D["out"] = nc.dram_tensor("out", [DM, ROWS], F32,
                              kind="ExternalOutput").ap()
    if DEBUG_DUMPS:
        for nm, shape in [("dbg_den", (16, ROWS)), ("dbg_ao", (P, ROWS))]:
            D[nm] = nc.dram_tensor(nm, list(shape), F32,
                                   kind="ExternalOutput").ap()
        D["dbg_echo"] = nc.dram_tensor("dbg_echo", [P, 1024], BF16,
                                       kind="ExternalOutput").ap()
        D["dbg_echo2"] = nc.dram_tensor("dbg_echo2", [P, 1024], BF16,
                                        kind="ExternalOutput").ap()
        for nm, shape in [("dbg_xb", (P, ROWS)), ("dbg_w", (P, 1024)),
                          ("dbg_q", (P, ROWS)), ("dbg_kt", (P, ROWS)),
                          ("dbg_ex", (P, 512)), ("dbg_vt", (P, VW)),
                          ("dbg_aob", (P, ROWS))]:
            D[nm] = nc.dram_tensor(nm, list(shape), BF16,
                                   kind="ExternalOutput").ap()
    with tile.TileContext(nc) as tc:
        _emit(nc, tc, D)
        D["_es"].close()
    nc.compile()
    _CACHE["nc"] = nc
    return nc


def _bf(a):
    return np.asarray(a, np.float32).astype(BF)


def _u32(a):
    """bf16 array -> uint32 view (works around bf16 input-transfer corruption)."""
    return np.ascontiguousarray(a).view(np.uint32)


def _make_in_maps(inputs):
    x = np.asarray(inputs["x"], np.float32)
    enc = np.asarray(inputs["enc_out"], np.float32)
    mask = np.asarray(inputs["look_ahead_mask"], np.float32)[0, 0]  # [T, T]
    shared = {w: _u32(_bf(inputs[w]))
              for w in ["wq1", "wk1", "wv1", "wo1", "wq2", "wk2", "wv2",
                        "wo2", "wh", "wout"]}
    vec_rows = [inputs[n] for n in
                ["bq1", "bk1", "bo1", "bq2", "bk2", "bo2", "bout",
                 "g1", "be1", "g2", "be2", "g3", "be3"]]
    vec_rows += list(np.asarray(inputs["bh"], np.float32).reshape(4, DM))
    shared["vecs"] = np.ascontiguousarray(
        np.stack([np.asarray(r, np.float32) for r in vec_rows]))
    shared["bvb"] = _u32(np.stack(
        [_bf(inputs["bv1"]).reshape(2, 512), _bf(inputs["bv2"]).reshape(2, 512)]))
    sel = np.zeros((16, NKT * P), np.float32)
    for hp in range(NKT):
        sel[2 * hp, hp * P:hp * P + DEPTH] = 1.0
        sel[2 * hp + 1, hp * P + DEPTH:(hp + 1) * P] = 1.0
    shared["sel"] = sel
    in_maps = []
    for i in range(N_CORES):
        sl = slice(i * TLOC, (i + 1) * TLOC)
        m = dict(shared)
        m["xTb"] = _u32(np.ascontiguousarray(
            x[:, sl, :].transpose(2, 0, 1).reshape(DM, ROWS)).astype(BF))
        m["encTb"] = _u32(np.ascontiguousarray(
            enc[:, sl, :].transpose(2, 0, 1).reshape(DM, ROWS)).astype(BF))
        a = np.ascontiguousarray(mask[sl, :].T * np.float32(-1e9))
        a = a.reshape(8, P, P)                       # [chunk, k_lo, q]
        mt = np.stack([np.concatenate([a[4 * g + c] for c in range(4)],
                                      axis=1) for g in range(2)])
        m["maskT"] = _u32(mt.astype(BF))
        in_maps.append(m)
    return in_maps


def _assemble(res):
    out = np.empty((B, T, DM), dtype=np.float32)
    for i in range(N_CORES):
        o = np.asarray(res.results[i]["out"])        # [DM, ROWS]
        out[:, i * TLOC:(i + 1) * TLOC, :] = \
            o.reshape(DM, B, TLOC).transpose(1, 2, 0)
    return out


def kernel(**inputs):
    nc = build()
    in_maps = _make_in_maps(inputs)
    res = run_bass_kernel_spmd(nc, in_maps, core_ids=list(range(N_CORES)))
    return _assemble(res)
